# Initial kernel scaffold
#
"""Trainium2 Bass kernel for a RoBERTa-style attention layer with
relative_key_query position bias (H=16, D=64, HID=1024, FF=4096, B=4, S=1024).

Sharding: 8 cores, zero-collective. Core c handles batch b=c//2, query-token
half `half=c%2` (rows l0=512*half .. l0+511). K/V are computed for the full
sequence on each core (duplicated within the batch pair); everything after
attention is token-parallel. The relative-position shear is done via DRAM
bounce buffers read back with linearly-sliding per-row offsets (legal strided
DMA APs with contiguous inner dim).

SPMD-uniformity: per-core differences enter only through input data:
 - hsq  : this core's 512 query rows of hidden_states
 - emb  : dist_emb[l0 : l0+1535]   (so in-kernel E-index = l_local - r + 1023)
 - embr : the same slice, rows reversed
"""
import numpy as np

H, D, HID, FF = 16, 64, 1024, 4096
S = 1024          # full sequence (keys)
L = 512           # query rows per core
B = 4
EPS = 1e-12

_cache = {}


def _build():
    import concourse.bass as bass
    import concourse.mybir as mybir
    import concourse.bacc as bacc
    import concourse.tile as tile
    from contextlib import ExitStack

    F32, BF16 = mybir.dt.float32, mybir.dt.bfloat16
    AF = mybir.ActivationFunctionType
    ALU = mybir.AluOpType

    nc = bacc.Bacc("TRN2", target_bir_lowering=False, debug=False, num_devices=8)

    dr = {}
    def din(name, shape):
        dr[name] = nc.dram_tensor(name, shape, F32, kind="ExternalInput")
        return dr[name]

    hs_d   = din("hs",   [S, HID])
    hsq_d  = din("hsq",  [L, HID])
    emb_d  = din("emb",  [1535, D])
    embr_d = din("embr", [1535, D])
    mask_d = din("mask", [1, S])
    hm_d   = din("hm",   [1, H])
    wq_d, bq_d = din("wq", [HID, HID]), din("bq", [1, HID])
    wk_d, bk_d = din("wk", [HID, HID]), din("bk", [1, HID])
    wv_d, bv_d = din("wv", [HID, HID]), din("bv", [1, HID])
    wo_d, bo_d = din("wo", [HID, HID]), din("bo", [1, HID])
    l1g_d, l1b_d = din("l1g", [1, HID]), din("l1b", [1, HID])
    wi_d, bi_d = din("wi", [HID, FF]), din("bi", [1, FF])
    wo2_d, bo2_d = din("wo2", [FF, HID]), din("bo2", [1, HID])
    l2g_d, l2b_d = din("l2g", [1, HID]), din("l2b", [1, HID])
    out_d = nc.dram_tensor("out", [L, HID], F32, kind="ExternalOutput")

    with tile.TileContext(nc) as tc:
        es_main = ExitStack()      # whole-kernel pools
        es_qkv = ExitStack()       # QT/KT/VA, closed after attention
        es_p1 = ExitStack()        # QKV-phase temps
        es_ctx = ExitStack()       # ctxT, closed after Wo-proj
        es_wi = ExitStack()        # Wi weights, closed after FFN1
        es_head = ExitStack()      # attention per-head temps
        es_tail = ExitStack()      # phase-3/4 tiles

        cst = es_main.enter_context(tc.tile_pool(name="cst", bufs=1))
        resid_p = es_main.enter_context(tc.tile_pool(name="residp", bufs=1))
        wob_p = es_main.enter_context(tc.tile_pool(name="wobp", bufs=1))

        # ---------- constants / small params ----------
        identb = cst.tile([128, 128], BF16, tag="identb")
        onesb = cst.tile([128, 128], BF16, tag="onesb")
        nc.gpsimd.memset(onesb[:], 1.0)
        nc.gpsimd.affine_select(identb[:], onesb[:], [[1, 128]], ALU.is_equal,
                                0.0, base=0, channel_multiplier=-1)

        def load_cols(name, src_d, n, tagp):
            """[1, n*128] DRAM row -> [128, n] SBUF (col j = elems j*128..j*128+127)."""
            t = cst.tile([128, n], F32, tag=tagp)
            for j in range(n):
                nc.sync.dma_start(t[:, j:j+1],
                                  bass.AP(src_d, j * 128, [[1, 128], [1, 1]]))
            return t

        qb_c = load_cols("qb", bq_d, 8, "qbc")
        kb_c = load_cols("kb", bk_d, 8, "kbc")
        bv_c = load_cols("bv", bv_d, 8, "bvc")
        mask_c = load_cols("mask", mask_d, 8, "maskc")
        bi_c = load_cols("bi", bi_d, 32, "bic")

        hm_sb = cst.tile([1, H], F32, tag="hmsb")
        nc.sync.dma_start(hm_sb[:], hm_d.ap())
        hmb = cst.tile([64, H], F32, tag="hmb")
        nc.gpsimd.partition_broadcast(hmb[:], hm_sb[:])

        def bcast_row(src_d, tagp):
            """[1, HID] DRAM -> [128, HID] bf16 broadcast rows."""
            r32 = cst.tile([1, HID], F32, tag=tagp + "f")
            nc.sync.dma_start(r32[:], src_d.ap())
            rb = cst.tile([1, HID], BF16, tag=tagp + "b")
            nc.vector.tensor_copy(rb[:], r32[:])
            t = cst.tile([128, HID], BF16, tag=tagp)
            nc.gpsimd.partition_broadcast(t[:], rb[:])
            return t

        bo_r = bcast_row(bo_d, "bor")
        l1g_r = bcast_row(l1g_d, "l1gr")
        l1b_r = bcast_row(l1b_d, "l1br")
        bo2_r = bcast_row(bo2_d, "bo2r")
        l2g_r = bcast_row(l2g_d, "l2gr")
        l2b_r = bcast_row(l2b_d, "l2br")

        # ET / ETr: [64, 1535] bf16 built by PE-transposing 128-row chunks
        ET = cst.tile([64, 1536], BF16, tag="et")
        ETr = cst.tile([64, 1536], BF16, tag="etr")
        with tc.tile_pool(name="embt", bufs=2) as embt, \
             tc.tile_pool(name="embps", bufs=2, space="PSUM") as embps:
            for (src, dst) in ((emb_d, ET), (embr_d, ETr)):
                for j in range(12):
                    rows = 128 if j < 11 else 127
                    tf = embt.tile([128, D], F32, tag="embf")
                    nc.sync.dma_start(tf[0:rows, :],
                                      bass.AP(src, j * 128 * D, [[D, rows], [1, D]]))
                    tb = embt.tile([128, D], BF16, tag="embb")
                    nc.vector.tensor_copy(tb[0:rows, :], tf[0:rows, :])
                    pt = embps.tile([64, 128], BF16, tag="embps")
                    nc.tensor.transpose(pt[:, 0:rows], tb[0:rows, :], identb[:])
                    nc.scalar.copy(dst[:, j*128:j*128+rows], pt[:, 0:rows])

        # ---------- residual rows (hsq f32) ----------
        resid = [resid_p.tile([128, HID], F32, tag=f"resid{t}") for t in range(4)]
        for t in range(4):
            nc.sync.dma_start(resid[t][:], hsq_d.ap()[t*128:(t+1)*128, :])

        # Wo bf16 (cast now; scheduler places loads wherever DMA is free)
        wob = [wob_p.tile([128, HID], BF16, tag=f"wob{i}") for i in range(8)]
        with tc.tile_pool(name="wotmp", bufs=2) as wotmp:
            for i in range(8):
                tf = wotmp.tile([128, HID], F32, tag="wotf")
                nc.sync.dma_start(tf[:], wo_d.ap()[i*128:(i+1)*128, :])
                nc.vector.tensor_copy(wob[i][:], tf[:])

        # ---------- phase 1: QKV projections ----------
        qkv = es_qkv.enter_context(tc.tile_pool(name="qkv", bufs=1))
        QT = [qkv.tile([128, L], BF16, tag=f"qt{d}") for d in range(8)]
        KT = [qkv.tile([128, S], BF16, tag=f"kt{d}") for d in range(8)]
        VA = [qkv.tile([128, 1040], BF16, tag=f"va{r}") for r in range(8)]

        p1 = es_p1.enter_context(tc.tile_pool(name="p1", bufs=2))
        wqkv_p = es_p1.enter_context(tc.tile_pool(name="wqkv", bufs=1))
        hst_p = es_p1.enter_context(tc.tile_pool(name="hst", bufs=1))
        p1ps = es_p1.enter_context(tc.tile_pool(name="p1ps", bufs=3, space="PSUM"))

        hsT = [hst_p.tile([128, S], BF16, tag=f"hst{i}") for i in range(8)]
        hsqT = [hst_p.tile([128, L], BF16, tag=f"hsqt{i}") for i in range(8)]

        for t in range(8):
            tf = p1.tile([128, HID], F32, tag="hsf")
            nc.sync.dma_start(tf[:], hs_d.ap()[t*128:(t+1)*128, :])
            tb = p1.tile([128, HID], BF16, tag="hsb")
            nc.vector.tensor_copy(tb[:], tf[:])
            for i in range(8):
                nc.sync.dma_start_transpose(hsT[i][:, t*128:(t+1)*128],
                                            tb[:, i*128:(i+1)*128])
        for t in range(4):
            tb = p1.tile([128, HID], BF16, tag="hsb")
            nc.vector.tensor_copy(tb[:], resid[t][:])
            for i in range(8):
                nc.sync.dma_start_transpose(hsqT[i][:, t*128:(t+1)*128],
                                            tb[:, i*128:(i+1)*128])

        wqb = [wqkv_p.tile([128, HID], BF16, tag=f"wqb{i}") for i in range(8)]
        wkb = [wqkv_p.tile([128, HID], BF16, tag=f"wkb{i}") for i in range(8)]
        wvb = [wqkv_p.tile([128, HID], BF16, tag=f"wvb{i}") for i in range(8)]
        for (wd, wt) in ((wq_d, wqb), (wk_d, wkb), (wv_d, wvb)):
            for i in range(8):
                tf = p1.tile([128, HID], F32, tag="wf")
                nc.sync.dma_start(tf[:], wd.ap()[i*128:(i+1)*128, :])
                nc.vector.tensor_copy(wt[i][:], tf[:])

        # QT[d,l] = sum_i wq[i,d] * hsqT[i,l]  (+bq per-partition on cast)
        for dt in range(8):
            ps = p1ps.tile([128, L], F32, tag="qkvps")
            for i in range(8):
                nc.tensor.matmul(ps[:], wqb[i][:, dt*128:(dt+1)*128], hsqT[i][:],
                                 start=(i == 0), stop=(i == 7))
            nc.scalar.activation(QT[dt][:], ps[:], AF.Identity, bias=qb_c[:, dt:dt+1])
        for dt in range(8):
            for rc in range(2):
                ps = p1ps.tile([128, L], F32, tag="qkvps")
                for i in range(8):
                    nc.tensor.matmul(ps[:], wkb[i][:, dt*128:(dt+1)*128],
                                     hsT[i][:, rc*512:(rc+1)*512],
                                     start=(i == 0), stop=(i == 7))
                nc.scalar.activation(KT[dt][:, rc*512:(rc+1)*512], ps[:],
                                     AF.Identity, bias=kb_c[:, dt:dt+1])
        # VA[r, h*65+e] = V0[r, h*64+e]; col h*65+64 stays 1.0   (bv folded later)
        for rt in range(8):
            nc.gpsimd.memset(VA[rt][:], 1.0)
            for nh in range(2):
                ps = p1ps.tile([128, L], F32, tag="qkvps")
                for i in range(8):
                    nc.tensor.matmul(ps[:], hsT[i][:, rt*128:(rt+1)*128],
                                     wvb[i][:, nh*512:(nh+1)*512],
                                     start=(i == 0), stop=(i == 7))
                dst = VA[rt][:, nh*520:(nh+1)*520].rearrange(
                    "p (j e) -> p j e", e=65)[:, :, 0:64]
                nc.scalar.copy(dst, ps[:].rearrange("p (j e) -> p j e", e=64))
        es_p1.close()

        # ---------- phase 2: attention (16 heads) ----------
        ctx_p = es_ctx.enter_context(tc.tile_pool(name="ctxp", bufs=1))
        ctxT = [ctx_p.tile([128, L], BF16, tag=f"ctxt{i}") for i in range(8)]

        wib_p = es_wi.enter_context(tc.tile_pool(name="wibp", bufs=1))
        wib = [wib_p.tile([128, FF], BF16, tag=f"wib{i}") for i in range(8)]
        with tc.tile_pool(name="witmp", bufs=2) as witmp:
            for i in range(8):
                for j in range(4):
                    tf = witmp.tile([128, 1024], F32, tag="wif")
                    nc.sync.dma_start(
                        tf[:], wi_d.ap()[i*128:(i+1)*128, j*1024:(j+1)*1024])
                    nc.vector.tensor_copy(wib[i][:, j*1024:(j+1)*1024], tf[:])

        hp = es_head.enter_context(tc.tile_pool(name="hp", bufs=2))
        qsh_p = es_head.enter_context(tc.tile_pool(name="qshp", bufs=5))
        band_ps = es_head.enter_context(tc.tile_pool(name="bandps", bufs=3, space="PSUM"))
        sc_ps = es_head.enter_context(tc.tile_pool(name="scps", bufs=2, space="PSUM"))
        tr_ps = es_head.enter_context(tc.tile_pool(name="trps", bufs=1, space="PSUM"))
        pv_ps = es_head.enter_context(tc.tile_pool(name="pvps", bufs=1, space="PSUM"))
        dram_p = es_head.enter_context(tc.tile_pool(name="dramp", bufs=3, space="DRAM"))

        for h in range(H):
            ht, poff = h // 2, (h % 2) * 64
            QTs = QT[ht][poff:poff+64, :]          # [64, 512]
            KTs = KT[ht][poff:poff+64, :]          # [64, 1024]

            # --- q-side sheared scores [l, r] per l-tile ---
            qsh = []
            for lt in range(4):
                vlo = 384 - lt * 128
                qe_sb = hp.tile([128, 1152], BF16, tag="qesb")
                for (c0, n) in ((0, 512), (512, 512), (1024, 127)):
                    ps = band_ps.tile([128, 512], F32, tag="bandps")
                    nc.tensor.matmul(ps[:, 0:n], QTs[:, lt*128:(lt+1)*128],
                                     ETr[:, vlo+c0:vlo+c0+n])
                    nc.scalar.copy(qe_sb[:, c0:c0+n], ps[:, 0:n])
                qed = dram_p.tile([128, 1152], BF16, tag="qed")
                nc.sync.dma_start(qed[:], qe_sb[:])
                q_t = qsh_p.tile([128, S], BF16, tag="qsh")
                nc.sync.dma_start(q_t[:], bass.AP(qed[:].tensor, 127,
                                                  [[1151, 128], [1, S]]))
                qsh.append(q_t)

            # --- k-side bands -> sheared scores_kT [r, l] per r-tile (consumed below) ---
            ksh = []
            for rt in range(8):
                ulo = 896 - rt * 128
                ke_sb = hp.tile([128, 640], BF16, tag="kesb")
                for (c0, n) in ((0, 512), (512, 127)):
                    ps = band_ps.tile([128, 512], F32, tag="bandps")
                    nc.tensor.matmul(ps[:, 0:n], KTs[:, rt*128:(rt+1)*128],
                                     ET[:, ulo+c0:ulo+c0+n])
                    nc.scalar.copy(ke_sb[:, c0:c0+n], ps[:, 0:n])
                ked = dram_p.tile([128, 640], BF16, tag="ked")
                nc.sync.dma_start(ked[:], ke_sb[:])
                k_t = hp.tile([128, L], BF16, tag="ksh")
                nc.sync.dma_start(k_t[:], bass.AP(ked[:].tensor, 127,
                                                  [[639, 128], [1, L]]))
                ksh.append(k_t)

            # --- scoresT per r-tile: qk + transpose(q-side) + k-side; exp; PV ---
            pv = pv_ps.tile([128, L], F32, tag="pvps")
            for rt in range(8):
                sc = sc_ps.tile([128, L], F32, tag="scps")
                nc.tensor.matmul(sc[:], KTs[:, rt*128:(rt+1)*128], QTs[:])
                tr = tr_ps.tile([128, L], BF16, tag="trps")
                for lt in range(4):
                    nc.tensor.transpose(tr[:, lt*128:(lt+1)*128],
                                        qsh[lt][:, rt*128:(rt+1)*128], identb[:])
                scf = hp.tile([128, L], F32, tag="scf")
                nc.vector.tensor_tensor(scf[:], sc[:], tr[:], op=ALU.add)
                nc.vector.tensor_tensor(scf[:], scf[:], ksh[rt][:], op=ALU.add)
                prb = hp.tile([128, L], BF16, tag="prb")
                nc.scalar.activation(prb[:], scf[:], AF.Exp,
                                     scale=0.125, bias=mask_c[:, rt:rt+1])
                nc.tensor.matmul(pv[0:65, :], VA[rt][:, h*65:(h+1)*65], prb[:],
                                 start=(rt == 0), stop=(rt == 7))

            rd = hp.tile([1, L], F32, tag="rd")
            nc.vector.reciprocal(rd[:], pv[64:65, :])
            rdh = hp.tile([1, L], F32, tag="rdh")
            nc.scalar.activation(rdh[:], rd[:], AF.Identity,
                                 scale=hm_sb[0:1, h:h+1])
            rdb = hp.tile([64, L], F32, tag="rdb")
            nc.gpsimd.partition_broadcast(rdb[:], rdh[:])
            bvh = hp.tile([64, 1], F32, tag="bvh")
            nc.vector.tensor_tensor(bvh[:], bv_c[poff:poff+64, ht:ht+1],
                                    hmb[0:64, h:h+1], op=ALU.mult)
            cslice = ctxT[ht][poff:poff+64, :]
            nc.vector.tensor_tensor(cslice, pv[0:64, :], rdb[:], op=ALU.mult)
            nc.scalar.activation(cslice, cslice, AF.Identity, bias=bvh[:])

        es_head.close()
        es_qkv.close()

        # ---------- phase 3: Wo projection + residual + LN1 ----------
        t3 = es_tail.enter_context(tc.tile_pool(name="t3", bufs=2))
        ao_p = es_tail.enter_context(tc.tile_pool(name="aop", bufs=1))
        aoT_p = es_tail.enter_context(tc.tile_pool(name="aotp", bufs=1))
        t3ps = es_tail.enter_context(tc.tile_pool(name="t3ps", bufs=3, space="PSUM"))

        attn_out = [ao_p.tile([128, HID], F32, tag=f"ao{t}") for t in range(4)]
        aoT = [aoT_p.tile([128, L], BF16, tag=f"aot{i}") for i in range(8)]

        def layer_norm(dst, x, g_row, b_row, scr_pool):
            """dst = LN(x) * g + b  over free axis (HID), x [128, HID] f32."""
            negsum = scr_pool.tile([128, 1], F32, tag="lnns")
            nc.vector.tensor_reduce(negsum[:], x, axis=mybir.AxisListType.X,
                                    op=ALU.add, negate=True)
            negmu = scr_pool.tile([128, 1], F32, tag="lnnm")
            nc.scalar.mul(negmu[:], negsum[:], 1.0 / HID)
            xc = scr_pool.tile([128, HID], F32, tag="lnxc")
            nc.scalar.activation(xc[:], x, AF.Identity, bias=negmu[:])
            sq = scr_pool.tile([128, HID], F32, tag="lnsq")
            ssq = scr_pool.tile([128, 1], F32, tag="lnssq")
            nc.scalar.activation(sq[:], xc[:], AF.Square, accum_out=ssq[:])
            sd = scr_pool.tile([128, 1], F32, tag="lnsd")
            nc.scalar.activation(sd[:], ssq[:], AF.Sqrt, scale=1.0 / HID, bias=EPS)
            rstd = scr_pool.tile([128, 1], F32, tag="lnrstd")
            nc.vector.reciprocal(rstd[:], sd[:])
            xn = scr_pool.tile([128, HID], F32, tag="lnxn")
            nc.scalar.activation(xn[:], xc[:], AF.Identity, scale=rstd[:])
            nc.vector.tensor_tensor(xn[:], xn[:], g_row[:], op=ALU.mult)
            nc.vector.tensor_tensor(dst, xn[:], b_row[:], op=ALU.add)

        for t in range(4):
            acc = t3.tile([128, HID], F32, tag="p3acc")
            for oc in range(2):
                ps = t3ps.tile([128, 512], F32, tag="p3ps")
                for hc in range(8):
                    nc.tensor.matmul(ps[:], ctxT[hc][:, t*128:(t+1)*128],
                                     wob[hc][:, oc*512:(oc+1)*512],
                                     start=(hc == 0), stop=(hc == 7))
                nc.vector.tensor_tensor(acc[:, oc*512:(oc+1)*512], ps[:],
                                        resid[t][:, oc*512:(oc+1)*512], op=ALU.add)
            nc.vector.tensor_tensor(acc[:], acc[:], bo_r[:], op=ALU.add)
            layer_norm(attn_out[t][:], acc[:], l1g_r, l1b_r, t3)
            aob = t3.tile([128, HID], BF16, tag="aob")
            nc.vector.tensor_copy(aob[:], attn_out[t][:])
            for i in range(8):
                nc.sync.dma_start_transpose(aoT[i][:, t*128:(t+1)*128],
                                            aob[:, i*128:(i+1)*128])
        es_ctx.close()

        # ---------- phase 4a: FFN1 (interT = gelu(WiT x + bi)) ----------
        it_p = es_tail.enter_context(tc.tile_pool(name="itp", bufs=1))
        interT = [it_p.tile([128, L], BF16, tag=f"it{f}") for f in range(32)]
        for ft in range(32):
            ps = t3ps.tile([128, 512], F32, tag="p3ps")
            for i in range(8):
                nc.tensor.matmul(ps[:], wib[i][:, ft*128:(ft+1)*128], aoT[i][:],
                                 start=(i == 0), stop=(i == 7))
            nc.scalar.activation(interT[ft][:], ps[:], AF.Gelu,
                                 bias=bi_c[:, ft:ft+1])
        es_wi.close()

        # ---------- phase 4b: FFN2 + residual + LN2 -> out ----------
        f2ps = es_tail.enter_context(tc.tile_pool(name="f2ps", bufs=8, space="PSUM"))
        w2t = es_tail.enter_context(tc.tile_pool(name="w2t", bufs=3))
        ops = [f2ps.tile([128, 512], F32, tag="f2ps") for _ in range(8)]
        for fc in range(32):
            wf = w2t.tile([128, HID], F32, tag="w2f")
            nc.sync.dma_start(wf[:], wo2_d.ap()[fc*128:(fc+1)*128, :])
            wb = w2t.tile([128, HID], BF16, tag="w2b")
            nc.vector.tensor_copy(wb[:], wf[:])
            for t in range(4):
                for oc in range(2):
                    nc.tensor.matmul(ops[t*2+oc][:],
                                     interT[fc][:, t*128:(t+1)*128],
                                     wb[:, oc*512:(oc+1)*512],
                                     start=(fc == 0), stop=(fc == 31))
        for t in range(4):
            acc = t3.tile([128, HID], F32, tag="p4acc")
            for oc in range(2):
                nc.vector.tensor_tensor(acc[:, oc*512:(oc+1)*512],
                                        ops[t*2+oc][:],
                                        attn_out[t][:, oc*512:(oc+1)*512],
                                        op=ALU.add)
            nc.vector.tensor_tensor(acc[:], acc[:], bo2_r[:], op=ALU.add)
            outf = t3.tile([128, HID], F32, tag="outf")
            layer_norm(outf[:], acc[:], l2g_r, l2b_r, t3)
            nc.sync.dma_start(out_d.ap()[t*128:(t+1)*128, :], outf[:])

        es_tail.close()
        es_main.close()

    nc.compile()
    return nc


def kernel(**inputs):
    from concourse.bass_utils import run_bass_kernel_spmd

    if "nc" not in _cache:
        _cache["nc"] = _build()
    nc = _cache["nc"]

    f = lambda a: np.ascontiguousarray(np.asarray(a, dtype=np.float32))
    hs = f(inputs["hidden_states"])          # [4, 1024, 1024]
    emb = f(inputs["dist_emb"])              # [2047, 64]
    mask = f(inputs["attention_mask"])       # [4, 1, 1, 1024]
    hm = f(inputs["head_mask"]).reshape(1, H)

    common = {
        "wq": f(inputs["Wq"]), "bq": f(inputs["bq"]).reshape(1, HID),
        "wk": f(inputs["Wk"]), "bk": f(inputs["bk"]).reshape(1, HID),
        "wv": f(inputs["Wv"]), "bv": f(inputs["bv"]).reshape(1, HID),
        "wo": f(inputs["Wo"]), "bo": f(inputs["bo"]).reshape(1, HID),
        "l1g": f(inputs["ln1_g"]).reshape(1, HID), "l1b": f(inputs["ln1_b"]).reshape(1, HID),
        "wi": f(inputs["Wi"]), "bi": f(inputs["bi"]).reshape(1, FF),
        "wo2": f(inputs["Wo2"]), "bo2": f(inputs["bo2"]).reshape(1, HID),
        "l2g": f(inputs["ln2_g"]).reshape(1, HID), "l2b": f(inputs["ln2_b"]).reshape(1, HID),
        "hm": hm,
    }
    in_maps = []
    for c in range(8):
        b, half = c // 2, c % 2
        l0 = 512 * half
        eslice = np.ascontiguousarray(emb[l0:l0 + 1535])
        in_maps.append(dict(common,
                            hs=np.ascontiguousarray(hs[b]),
                            hsq=np.ascontiguousarray(hs[b, l0:l0 + L]),
                            emb=eslice,
                            embr=np.ascontiguousarray(eslice[::-1]),
                            mask=np.ascontiguousarray(mask[b, 0, 0]).reshape(1, S)))

    res = run_bass_kernel_spmd(nc, in_maps, core_ids=list(range(8)))
    out = np.zeros((B, S, HID), np.float32)
    for c in range(8):
        b, half = c // 2, c % 2
        out[b, half * L:(half + 1) * L] = res.results[c]["out"]
    return out


# revision 17
# speedup vs baseline: 1.1334x; 1.1334x over previous
"""Trainium2 Bass kernel for a RoBERTa-style attention layer with
relative_key_query position bias (H=16, D=64, HID=1024, FF=4096, B=4, S=1024).

Sharding: 8 cores, zero-collective. Core c handles batch b=c//2, query-token
half `half=c%2` (rows l0=512*half .. l0+511). K/V are computed for the full
sequence on each core (duplicated within the batch pair); everything after
attention is token-parallel, so per-core outputs concatenate on the host.
The relative-position shear runs through DRAM bounce buffers read back with
per-row sliding offsets (strided DMA APs with contiguous inner dim).

SPMD-uniformity: per-core differences enter only through input data:
 - hsq  : this core's 512 query rows of hidden_states
 - emb  : dist_emb[l0 : l0+1535]   (so in-kernel E-index = l_local - r + 1023)
 - embr : the same slice, rows reversed
"""
import numpy as np

H, D, HID, FF = 16, 64, 1024, 4096
S = 1024          # full sequence (keys)
L = 512           # query rows per core
B = 4
EPS = 1e-12

_cache = {}


def _build():
    import concourse.bass as bass
    import concourse.mybir as mybir
    import concourse.bacc as bacc
    import concourse.tile as tile

    F32, BF16 = mybir.dt.float32, mybir.dt.bfloat16
    AF = mybir.ActivationFunctionType
    ALU = mybir.AluOpType

    nc = bacc.Bacc("TRN2", target_bir_lowering=False, debug=False, num_devices=8)

    def din(name, shape):
        return nc.dram_tensor(name, shape, F32, kind="ExternalInput")

    hs_d   = din("hs",   [S, HID])
    hsq_d  = din("hsq",  [L, HID])
    emb_d  = din("emb",  [1535, D])
    embr_d = din("embr", [1535, D])
    mask_d = din("mask", [1, S])
    hm_d   = din("hm",   [1, H])
    wq_d, bq_d = din("wq", [HID, HID]), din("bq", [1, HID])
    wk_d, bk_d = din("wk", [HID, HID]), din("bk", [1, HID])
    wv_d, bv_d = din("wv", [HID, HID]), din("bv", [1, HID])
    wo_d, bo_d = din("wo", [HID, HID]), din("bo", [1, HID])
    l1g_d, l1b_d = din("l1g", [1, HID]), din("l1b", [1, HID])
    wi_d, bi_d = din("wi", [HID, FF]), din("bi", [1, FF])
    wo2_d, bo2_d = din("wo2", [FF, HID]), din("bo2", [1, HID])
    l2g_d, l2b_d = din("l2g", [1, HID]), din("l2b", [1, HID])
    out_d = nc.dram_tensor("out", [L, HID], F32, kind="ExternalOutput")

    with tile.TileContext(nc) as tc, \
         tc.tile_pool(name="cst", bufs=1) as cst, \
         tc.tile_pool(name="ctxp", bufs=1) as ctx_p, \
         tc.tile_pool(name="residp", bufs=1) as resid_p:

        # ---------- constants / small params ----------
        identb = cst.tile([128, 128], BF16, tag="identb")
        onesb = cst.tile([128, 128], BF16, tag="onesb")
        nc.gpsimd.memset(onesb[:], 1.0)
        nc.gpsimd.affine_select(identb[:], onesb[:], [[1, 128]], ALU.is_equal,
                                0.0, base=0, channel_multiplier=-1)

        def load_cols(src_d, n, tagp):
            """[1, n*128] DRAM row -> [128, n] SBUF (col j = elems j*128..+127)."""
            t = cst.tile([128, n], F32, name=tagp, tag=tagp)
            for j in range(n):
                nc.sync.dma_start(t[:, j:j+1],
                                  bass.AP(src_d, j * 128, [[1, 128], [1, 1]]))
            return t

        qb_c = load_cols(bq_d, 8, "qbc")
        kb_c = load_cols(bk_d, 8, "kbc")
        mask_c = load_cols(mask_d, 8, "maskc")
        bi_c = load_cols(bi_d, 32, "bic")

        # bv as [64, 16]: col h = bv[h*64 : (h+1)*64]  (base-0 for every head)
        bv_c = cst.tile([64, H], F32, tag="bvc")
        for h in range(H):
            nc.sync.dma_start(bv_c[:, h:h+1],
                              bass.AP(bv_d, h * 64, [[1, 64], [1, 1]]))

        eps_c = cst.tile([128, 1], F32, tag="epsc")
        nc.gpsimd.memset(eps_c[:], EPS)

        hm_sb = cst.tile([1, H], F32, tag="hmsb")
        nc.sync.dma_start(hm_sb[:], hm_d.ap())
        hmb = cst.tile([128, H], F32, tag="hmb")
        nc.gpsimd.partition_broadcast(hmb[:], hm_sb[:])

        # ET / ETr: [64, 1535] bf16 built by PE-transposing 128-row chunks
        ET = cst.tile([64, 1536], BF16, tag="et")
        ETr = cst.tile([64, 1536], BF16, tag="etr")
        with tc.tile_pool(name="embt", bufs=2) as embt, \
             tc.tile_pool(name="embps", bufs=2, space="PSUM") as embps:
            for (src, dst) in ((emb_d, ET), (embr_d, ETr)):
                for j in range(12):
                    rows = 128 if j < 11 else 127
                    tf = embt.tile([128, D], F32, tag="embf")
                    nc.sync.dma_start(tf[0:rows, :],
                                      bass.AP(src, j * 128 * D, [[D, rows], [1, D]]))
                    tb = embt.tile([128, D], BF16, tag="embb")
                    nc.vector.tensor_copy(tb[0:rows, :], tf[0:rows, :])
                    pt = embps.tile([64, 128], BF16, tag="embps")
                    nc.tensor.transpose(pt[:, 0:rows], tb[0:rows, :],
                                        identb[0:rows, 0:rows])
                    nc.scalar.copy(dst[:, j*128:j*128+rows], pt[:, 0:rows])

        # residual rows (hsq f32), live through LN1
        resid = [resid_p.tile([128, HID], F32, name=f"resid{t}", tag=f"resid{t}")
                 for t in range(4)]
        for t in range(4):
            nc.sync.dma_start(resid[t][:], hsq_d.ap()[t*128:(t+1)*128, :])

        # ctxT accumulates attention output per head, consumed in phase 3
        ctxT = [ctx_p.tile([128, L], BF16, name=f"ctxt{i}", tag=f"ctxt{i}")
                for i in range(8)]

        # ---------- phases 1-2 under the qkv pool scope ----------
        with tc.tile_pool(name="qkv", bufs=1) as qkv:
            QT = [qkv.tile([128, L], BF16, name=f"qt{d}", tag=f"qt{d}") for d in range(8)]
            KT = [qkv.tile([128, S], BF16, name=f"kt{d}", tag=f"kt{d}") for d in range(8)]
            VA = [qkv.tile([128, 1040], BF16, name=f"va{r}", tag=f"va{r}") for r in range(8)]

            # ----- phase 1: QKV projections -----
            with tc.tile_pool(name="p1", bufs=2) as p1, \
                 tc.tile_pool(name="wqkv", bufs=1) as wqkv_p, \
                 tc.tile_pool(name="hst", bufs=1) as hst_p, \
                 tc.tile_pool(name="p1ps", bufs=3, space="PSUM") as p1ps:

                hsT = [hst_p.tile([128, S], BF16, name=f"hst{i}", tag=f"hst{i}")
                       for i in range(8)]
                hsqT = [hst_p.tile([128, L], BF16, name=f"hsqt{i}", tag=f"hsqt{i}")
                        for i in range(8)]

                for t in range(8):
                    tf = p1.tile([128, HID], F32, tag="hsf")
                    nc.sync.dma_start(tf[:], hs_d.ap()[t*128:(t+1)*128, :])
                    tb = p1.tile([128, HID], BF16, tag="hsb")
                    nc.vector.tensor_copy(tb[:], tf[:])
                    for i in range(8):
                        nc.sync.dma_start_transpose(hsT[i][:, t*128:(t+1)*128],
                                                    tb[:, i*128:(i+1)*128])
                for t in range(4):
                    tb = p1.tile([128, HID], BF16, tag="hsb")
                    nc.vector.tensor_copy(tb[:], resid[t][:])
                    for i in range(8):
                        nc.sync.dma_start_transpose(hsqT[i][:, t*128:(t+1)*128],
                                                    tb[:, i*128:(i+1)*128])

                wqb = [wqkv_p.tile([128, HID], BF16, name=f"wqb{i}", tag=f"wqb{i}")
                       for i in range(8)]
                wkb = [wqkv_p.tile([128, HID], BF16, name=f"wkb{i}", tag=f"wkb{i}")
                       for i in range(8)]
                wvb = [wqkv_p.tile([128, HID], BF16, name=f"wvb{i}", tag=f"wvb{i}")
                       for i in range(8)]
                for (wd, wt) in ((wq_d, wqb), (wk_d, wkb), (wv_d, wvb)):
                    for i in range(8):
                        tf = p1.tile([128, HID], F32, tag="wf")
                        nc.sync.dma_start(tf[:], wd.ap()[i*128:(i+1)*128, :])
                        nc.vector.tensor_copy(wt[i][:], tf[:])

                # QT[d,l] = sum_i wq[i,d] * hsqT[i,l]  (+bq per-partition on cast)
                for dt in range(8):
                    ps = p1ps.tile([128, L], F32, tag="qkvps")
                    for i in range(8):
                        nc.tensor.matmul(ps[:], wqb[i][:, dt*128:(dt+1)*128],
                                         hsqT[i][:], start=(i == 0), stop=(i == 7))
                    nc.scalar.activation(QT[dt][:], ps[:], AF.Identity,
                                         bias=qb_c[:, dt:dt+1])
                for dt in range(8):
                    for rc in range(2):
                        ps = p1ps.tile([128, L], F32, tag="qkvps")
                        for i in range(8):
                            nc.tensor.matmul(ps[:], wkb[i][:, dt*128:(dt+1)*128],
                                             hsT[i][:, rc*512:(rc+1)*512],
                                             start=(i == 0), stop=(i == 7))
                        nc.scalar.activation(KT[dt][:, rc*512:(rc+1)*512], ps[:],
                                             AF.Identity, bias=kb_c[:, dt:dt+1])
                # VA[r, h*65+e] = V0[r, h*64+e]; col h*65+64 stays 1.0 (bv folded later)
                for rt in range(8):
                    nc.gpsimd.memset(VA[rt][:], 1.0)
                    for nh in range(2):
                        ps = p1ps.tile([128, L], F32, tag="qkvps")
                        for i in range(8):
                            nc.tensor.matmul(ps[:], hsT[i][:, rt*128:(rt+1)*128],
                                             wvb[i][:, nh*512:(nh+1)*512],
                                             start=(i == 0), stop=(i == 7))
                        dst = VA[rt][:, nh*520:(nh+1)*520].rearrange(
                            "p (j e) -> p j e", e=65)[:, :, 0:64]
                        nc.scalar.copy(dst, ps[:].rearrange("p (j e) -> p j e", e=64))

            # ----- phase 2: attention, 16 heads -----
            with tc.tile_pool(name="hp", bufs=2) as hp, \
                 tc.tile_pool(name="qshp", bufs=5) as qsh_p, \
                 tc.tile_pool(name="bandps", bufs=3, space="PSUM") as band_ps, \
                 tc.tile_pool(name="scps", bufs=2, space="PSUM") as sc_ps, \
                 tc.tile_pool(name="trps", bufs=2, space="PSUM") as tr_ps, \
                 tc.tile_pool(name="pvps", bufs=1, space="PSUM") as pv_ps, \
                 tc.tile_pool(name="dramp", bufs=3, space="DRAM") as dram_p:

                for h in range(H):
                    ht, poff = h // 2, (h % 2) * 64
                    if h % 2 == 0:
                        QTs = QT[ht][0:64, :]          # [64, 512], base 0
                        KTs = KT[ht][0:64, :]          # [64, 1024], base 0
                    else:
                        # matmul needs equal operand base partitions; copy the
                        # odd head's rows 64:128 down to base 0 via DMA.
                        qto = hp.tile([64, L], BF16, tag="qto")
                        nc.sync.dma_start(qto[:], QT[ht][64:128, :])
                        kto = hp.tile([64, S], BF16, tag="kto")
                        nc.sync.dma_start(kto[:], KT[ht][64:128, :])
                        QTs, KTs = qto[:], kto[:]

                    # --- q-side sheared scores [l, r] per l-tile ---
                    qsh = []
                    for lt in range(4):
                        vlo = 384 - lt * 128
                        qe_sb = hp.tile([128, 1152], BF16, tag="qesb")
                        for (c0, n) in ((0, 512), (512, 512), (1024, 127)):
                            ps = band_ps.tile([128, 512], F32, tag="bandps")
                            nc.tensor.matmul(ps[:, 0:n], QTs[:, lt*128:(lt+1)*128],
                                             ETr[:, vlo+c0:vlo+c0+n])
                            nc.scalar.copy(qe_sb[:, c0:c0+n], ps[:, 0:n])
                        qed = dram_p.tile([128, 1152], BF16, tag="qed")
                        nc.sync.dma_start(qed[:], qe_sb[:])
                        q_t = qsh_p.tile([128, S], BF16, tag="qsh")
                        nc.sync.dma_start(q_t[:], bass.AP(qed[:].tensor, 127,
                                                          [[1151, 128], [1, S]]))
                        qsh.append(q_t)

                    # --- per r-tile: k-side band+shear; scoresT; exp; PV ---
                    pv = pv_ps.tile([128, L], F32, tag="pvps")
                    for rt in range(8):
                        ulo = 896 - rt * 128
                        ke_sb = hp.tile([128, 640], BF16, tag="kesb")
                        for (c0, n) in ((0, 512), (512, 127)):
                            ps = band_ps.tile([128, 512], F32, tag="bandps")
                            nc.tensor.matmul(ps[:, 0:n], KTs[:, rt*128:(rt+1)*128],
                                             ET[:, ulo+c0:ulo+c0+n])
                            nc.scalar.copy(ke_sb[:, c0:c0+n], ps[:, 0:n])
                        ked = dram_p.tile([128, 640], BF16, tag="ked")
                        nc.sync.dma_start(ked[:], ke_sb[:])
                        ksh = hp.tile([128, L], BF16, tag="ksh")
                        nc.sync.dma_start(ksh[:], bass.AP(ked[:].tensor, 127,
                                                          [[639, 128], [1, L]]))

                        sc = sc_ps.tile([128, L], F32, tag="scps")
                        nc.tensor.matmul(sc[:], KTs[:, rt*128:(rt+1)*128], QTs[:])
                        tr = tr_ps.tile([128, L], BF16, tag="trps")
                        for lt in range(4):
                            nc.tensor.transpose(tr[:, lt*128:(lt+1)*128],
                                                qsh[lt][:, rt*128:(rt+1)*128],
                                                identb[:])
                        scf = hp.tile([128, L], F32, tag="scf")
                        # DVE may read at most one PSUM operand per op:
                        nc.vector.tensor_tensor(scf[:], tr[:], ksh[:], op=ALU.add)
                        nc.vector.tensor_tensor(scf[:], scf[:], sc[:], op=ALU.add)
                        prb = hp.tile([128, L], BF16, tag="prb")
                        nc.scalar.activation(prb[:], scf[:], AF.Exp,
                                             scale=0.125, bias=mask_c[:, rt:rt+1])
                        nc.tensor.matmul(pv[0:65, :], VA[rt][:, h*65:(h+1)*65],
                                         prb[:], start=(rt == 0), stop=(rt == 7))

                    # normalize (pattern validated in exp4: base-0 recip tile,
                    # gpsimd broadcast from partition 0)
                    pvs = hp.tile([65, L], F32, tag="pvs")
                    nc.scalar.copy(pvs[:], pv[0:65, :])
                    rd = hp.tile([1, L], F32, tag="rd")
                    nc.vector.reciprocal(rd[:], pvs[64:65, :])
                    nc.scalar.activation(rd[:], rd[:], AF.Identity,
                                         scale=hmb[0:1, h:h+1])
                    rdb = hp.tile([64, L], F32, tag="rdb")
                    nc.gpsimd.partition_broadcast(rdb[:], rd[:])
                    bvh = hp.tile([64, 1], F32, tag="bvh")
                    nc.vector.tensor_tensor(bvh[:], bv_c[:, h:h+1],
                                            hmb[0:64, h:h+1], op=ALU.mult)
                    ctxh = hp.tile([64, L], BF16, tag="ctxh")
                    nc.vector.tensor_tensor(ctxh[:], pvs[0:64, :], rdb[:],
                                            op=ALU.mult)
                    nc.scalar.activation(ctxh[:], ctxh[:], AF.Identity, bias=bvh[:])
                    nc.sync.dma_start(ctxT[ht][poff:poff+64, :], ctxh[:])

        # ---------- phases 3-4 ----------
        def layer_norm(dst, x, g_row, b_row, scr_pool):
            """dst = LN(x) * g + b over the free axis; x [128, HID] f32 is
            clobbered (reused as square scratch)."""
            negsum = scr_pool.tile([128, 1], F32, name="lnns", tag="lnns")
            nc.vector.tensor_reduce(negsum[:], x, axis=mybir.AxisListType.X,
                                    op=ALU.add, negate=True)
            negmu = scr_pool.tile([128, 1], F32, name="lnnm", tag="lnnm")
            nc.scalar.mul(negmu[:], negsum[:], 1.0 / HID)
            xc = scr_pool.tile([128, HID], F32, name="lnxc", tag="lnxc")
            nc.scalar.activation(xc[:], x, AF.Identity, bias=negmu[:])
            ssq = scr_pool.tile([128, 1], F32, name="lnssq", tag="lnssq")
            nc.scalar.activation(x, xc[:], AF.Square, accum_out=ssq[:])
            sd = scr_pool.tile([128, 1], F32, name="lnsd", tag="lnsd")
            nc.scalar.activation(sd[:], ssq[:], AF.Sqrt, scale=1.0 / HID,
                                 bias=eps_c[:])
            rstd = scr_pool.tile([128, 1], F32, name="lnrstd", tag="lnrstd")
            nc.vector.reciprocal(rstd[:], sd[:])
            nc.scalar.activation(xc[:], xc[:], AF.Identity, scale=rstd[:])
            nc.vector.tensor_tensor(xc[:], xc[:], g_row[:], op=ALU.mult)
            nc.vector.tensor_tensor(dst, xc[:], b_row[:], op=ALU.add)

        def bcast_row(pool, src_d, tagp):
            r32 = pool.tile([1, HID], F32, name=tagp + "f", tag=tagp + "f")
            nc.sync.dma_start(r32[:], src_d.ap())
            rb = pool.tile([1, HID], BF16, name=tagp + "b", tag=tagp + "b")
            nc.vector.tensor_copy(rb[:], r32[:])
            t = pool.tile([128, HID], BF16, name=tagp, tag=tagp)
            nc.gpsimd.partition_broadcast(t[:], rb[:])
            return t

        with tc.tile_pool(name="aop", bufs=1) as ao_p, \
             tc.tile_pool(name="aotp", bufs=1) as aoT_p:
            attn_out = [ao_p.tile([128, HID], F32, name=f"ao{t}", tag=f"ao{t}")
                        for t in range(4)]
            aoT = [aoT_p.tile([128, L], BF16, name=f"aot{i}", tag=f"aot{i}")
                   for i in range(8)]

            # ----- phase 3: Wo projection + residual + LN1 -----
            with tc.tile_pool(name="p3", bufs=1) as p3, \
                 tc.tile_pool(name="p3w", bufs=2) as p3w, \
                 tc.tile_pool(name="wobp", bufs=1) as wob_p, \
                 tc.tile_pool(name="p3ps", bufs=3, space="PSUM") as t3ps:
                bo_r = bcast_row(p3, bo_d, "bor")
                l1g_r = bcast_row(p3, l1g_d, "l1gr")
                l1b_r = bcast_row(p3, l1b_d, "l1br")
                wob = [wob_p.tile([128, HID], BF16, name=f"wob{i}", tag=f"wob{i}")
                       for i in range(8)]
                for i in range(8):
                    tf = p3w.tile([128, HID], F32, tag="wotf")
                    nc.sync.dma_start(tf[:], wo_d.ap()[i*128:(i+1)*128, :])
                    nc.vector.tensor_copy(wob[i][:], tf[:])

                for t in range(4):
                    acc = p3.tile([128, HID], F32, tag="p3acc")
                    for oc in range(2):
                        ps = t3ps.tile([128, 512], F32, tag="p3ps")
                        for hc in range(8):
                            nc.tensor.matmul(ps[:], ctxT[hc][:, t*128:(t+1)*128],
                                             wob[hc][:, oc*512:(oc+1)*512],
                                             start=(hc == 0), stop=(hc == 7))
                        nc.vector.tensor_tensor(acc[:, oc*512:(oc+1)*512], ps[:],
                                                resid[t][:, oc*512:(oc+1)*512],
                                                op=ALU.add)
                    nc.vector.tensor_tensor(acc[:], acc[:], bo_r[:], op=ALU.add)
                    layer_norm(attn_out[t][:], acc[:], l1g_r, l1b_r, p3)
                    aob = p3.tile([128, HID], BF16, tag="aob")
                    nc.vector.tensor_copy(aob[:], attn_out[t][:])
                    for i in range(8):
                        nc.sync.dma_start_transpose(aoT[i][:, t*128:(t+1)*128],
                                                    aob[:, i*128:(i+1)*128])

            # ----- phase 4a: FFN1 (interT = gelu(WiT x + bi)) -----
            with tc.tile_pool(name="itp", bufs=1) as it_p:
                interT = [it_p.tile([128, L], BF16, name=f"it{f}", tag=f"it{f}")
                          for f in range(32)]
                with tc.tile_pool(name="wibp", bufs=1) as wib_p, \
                     tc.tile_pool(name="witmp", bufs=2) as witmp, \
                     tc.tile_pool(name="f1ps", bufs=3, space="PSUM") as f1ps:
                    wib = [wib_p.tile([128, FF], BF16, name=f"wib{i}", tag=f"wib{i}")
                           for i in range(8)]
                    for i in range(8):
                        for j in range(4):
                            tf = witmp.tile([128, 1024], F32, tag="wif")
                            nc.sync.dma_start(
                                tf[:], wi_d.ap()[i*128:(i+1)*128, j*1024:(j+1)*1024])
                            nc.vector.tensor_copy(wib[i][:, j*1024:(j+1)*1024],
                                                  tf[:])
                    for ft in range(32):
                        ps = f1ps.tile([128, 512], F32, tag="f1ps")
                        for i in range(8):
                            nc.tensor.matmul(ps[:], wib[i][:, ft*128:(ft+1)*128],
                                             aoT[i][:], start=(i == 0), stop=(i == 7))
                        nc.scalar.activation(interT[ft][:], ps[:], AF.Gelu,
                                             bias=bi_c[:, ft:ft+1])

                # ----- phase 4b: FFN2 + residual + LN2 -> out -----
                with tc.tile_pool(name="p4", bufs=1) as p4, \
                     tc.tile_pool(name="w2t", bufs=3) as w2t, \
                     tc.tile_pool(name="f2ps", bufs=8, space="PSUM") as f2ps:
                    bo2_r = bcast_row(p4, bo2_d, "bo2r")
                    l2g_r = bcast_row(p4, l2g_d, "l2gr")
                    l2b_r = bcast_row(p4, l2b_d, "l2br")
                    ops = [f2ps.tile([128, 512], F32, name=f"f2ps{j}", tag="f2ps")
                           for j in range(8)]
                    for fc in range(32):
                        wf = w2t.tile([128, HID], F32, tag="w2f")
                        nc.sync.dma_start(wf[:], wo2_d.ap()[fc*128:(fc+1)*128, :])
                        wb = w2t.tile([128, HID], BF16, tag="w2b")
                        nc.vector.tensor_copy(wb[:], wf[:])
                        for t in range(4):
                            for oc in range(2):
                                nc.tensor.matmul(ops[t*2+oc][:],
                                                 interT[fc][:, t*128:(t+1)*128],
                                                 wb[:, oc*512:(oc+1)*512],
                                                 start=(fc == 0), stop=(fc == 31))
                    for t in range(4):
                        acc = p4.tile([128, HID], F32, tag="p4acc")
                        for oc in range(2):
                            nc.vector.tensor_tensor(acc[:, oc*512:(oc+1)*512],
                                                    ops[t*2+oc][:],
                                                    attn_out[t][:, oc*512:(oc+1)*512],
                                                    op=ALU.add)
                        nc.vector.tensor_tensor(acc[:], acc[:], bo2_r[:], op=ALU.add)
                        outf = p4.tile([128, HID], F32, tag="outf")
                        layer_norm(outf[:], acc[:], l2g_r, l2b_r, p4)
                        nc.sync.dma_start(out_d.ap()[t*128:(t+1)*128, :], outf[:])

    nc.compile()
    return nc


def kernel(**inputs):
    from concourse.bass_utils import run_bass_kernel_spmd

    if "nc" not in _cache:
        _cache["nc"] = _build()
    nc = _cache["nc"]

    f = lambda a: np.ascontiguousarray(np.asarray(a, dtype=np.float32))
    hs = f(inputs["hidden_states"])          # [4, 1024, 1024]
    emb = f(inputs["dist_emb"])              # [2047, 64]
    mask = f(inputs["attention_mask"])       # [4, 1, 1, 1024]
    hm = f(inputs["head_mask"]).reshape(1, H)

    common = {
        "wq": f(inputs["Wq"]), "bq": f(inputs["bq"]).reshape(1, HID),
        "wk": f(inputs["Wk"]), "bk": f(inputs["bk"]).reshape(1, HID),
        "wv": f(inputs["Wv"]), "bv": f(inputs["bv"]).reshape(1, HID),
        "wo": f(inputs["Wo"]), "bo": f(inputs["bo"]).reshape(1, HID),
        "l1g": f(inputs["ln1_g"]).reshape(1, HID),
        "l1b": f(inputs["ln1_b"]).reshape(1, HID),
        "wi": f(inputs["Wi"]), "bi": f(inputs["bi"]).reshape(1, FF),
        "wo2": f(inputs["Wo2"]), "bo2": f(inputs["bo2"]).reshape(1, HID),
        "l2g": f(inputs["ln2_g"]).reshape(1, HID),
        "l2b": f(inputs["ln2_b"]).reshape(1, HID),
        "hm": hm,
    }
    in_maps = []
    for c in range(8):
        b, half = c // 2, c % 2
        l0 = 512 * half
        eslice = np.ascontiguousarray(emb[l0:l0 + 1535])
        in_maps.append(dict(common,
                            hs=np.ascontiguousarray(hs[b]),
                            hsq=np.ascontiguousarray(hs[b, l0:l0 + L]),
                            emb=eslice,
                            embr=np.ascontiguousarray(eslice[::-1]),
                            mask=np.ascontiguousarray(mask[b, 0, 0]).reshape(1, S)))

    globals()["_last_in_maps"] = in_maps
    res = run_bass_kernel_spmd(nc, in_maps, core_ids=list(range(8)))
    out = np.zeros((B, S, HID), np.float32)
    for c in range(8):
        b, half = c // 2, c % 2
        out[b, half * L:(half + 1) * L] = res.results[c]["out"]
    return out
